# revision 1
# baseline (speedup 1.0000x reference)
"""Trainium2 Bass kernel for nn_Conv2d_mvm (PUMA bit-sliced crossbar conv emulation).

Math identity
-------------
The reference emulates an analog crossbar MVM: inputs become 16-bit
two's-complement bit-streams, weights become 2-bit slices of the 16-bit
magnitudes of their pos/neg parts, and ADC = clip(round(analog), 0, 511).
Each analog column sum is at most 128*3 = 384 < 511 and every quantity is a
small exact integer held in f32, so the ADC is the identity and the whole
pipeline is linear in the bits/slices. Shift-add therefore reconstructs

    out[p, c] = quant( (x_int[p, :] . w_int[c, :]) / 2^24 )

with x_int = round(patch * 2^12) (int16 wrap),
w_int = clip(round(relu(w)*2^12), 0, 65535) - clip(round(relu(-w)*2^12), 0, 65535),
quant(v) = clip(round(v * 2^12), -2^15, 2^15-1) / 2^12  (round-half-even).

Device kernel
-------------
Data-parallel over the P = 1024 output pixels: each of 8 cores computes 128
pixels (half of one batch image) against the replicated [L=576, Cout=128]
integer weight matrix.

The PE's fp32 matmul is double-pumped (LOW/HIGH passes), so the integer
matmul is run in fp16 instead, which is exact here: |w_int| < 2048 fits
fp16's 11-bit mantissa, and x_int = 256*xh + xl splits into two fp16-exact
factors. The split folds into the contraction dimension,

    acc[p,c] = sum_l (256*xh[l,p]) * w[l,c] + xl[l,p] * w[l,c],

run as two 5-tile accumulation groups (hi block then lo block, each zero
padded to 640 rows) that share the same 5 weight k-tiles: 10 single-pass
fp16 matmuls into one PSUM bank. A 3-instruction DVE epilogue applies the
fixed-point quantizer (RNE via the +1.5*2^23 magic constant, clip, rescale).
Should some input exceed the fp16-exact ranges, kernel() falls back to an
fp32 program (5 double-pumped matmuls over k = 640).
"""

import numpy as np

# Problem constants (hardcoded: kernel.py must be self-contained).
B, CIN, H, W = 4, 64, 16, 16
COUT = 128
K, PAD = 3, 1
IF = 12           # input frac bits
WF = 12           # weight frac bits
ACM_FRAC = 12
L = CIN * K * K   # 576
N_CORES = 8
ROWS_PER_CORE = H // 2            # 8 pixel rows per core
PIX_PER_CORE = ROWS_PER_CORE * W  # 128
KTW = 5                           # weight k-tiles (640 = 5*128, zero-padded)
KTX = 2 * KTW                     # fp16 x k-tiles: 5 hi-part + 5 lo-part
KT32 = 5                          # fp32 k-tiles (640 = 5*128, zero-padded)

_CACHE = {}

_MAGIC = float(np.float32(1.5 * 2 ** 23))  # f32 RNE rounding constant
_INV_Q = 1.0 / (1 << ACM_FRAC)
_LO = float(-(1 << 15))
_HI = float((1 << 15) - 1)


def _epilogue_and_out(nc, mybir, pool, acc, out):
    """q = clip(round(acc / 2^12), -2^15, 2^15-1) / 2^12, then store.

    Runs as two independent pixel-half chains so the first half's store
    (a contiguous DRAM block) issues while the second half still quantizes.
    """
    res = pool.tile([PIX_PER_CORE, COUT], mybir.dt.float32, name="res")
    nc.vector.tensor_scalar(res[:, :], acc[:, :], _INV_Q, _MAGIC,
                            op0=mybir.AluOpType.mult,
                            op1=mybir.AluOpType.add)
    nc.vector.tensor_scalar(res[:, :], res[:, :], _MAGIC, _LO,
                            op0=mybir.AluOpType.subtract,
                            op1=mybir.AluOpType.max)
    nc.vector.tensor_scalar(res[:, :], res[:, :], _HI, _INV_Q,
                            op0=mybir.AluOpType.min,
                            op1=mybir.AluOpType.mult)
    half = PIX_PER_CORE // 2
    nc.sync.dma_start(out[0:half, :], res[0:half, :])
    nc.scalar.dma_start(out[half:PIX_PER_CORE, :], res[half:PIX_PER_CORE, :])


def _build_fp16_program():
    """10 single-pass fp16 matmuls (5 hi-part + 5 lo-part k-tiles sharing the
    same 5 weight tiles); inputs are pre-tiled, zero-padded on the host."""
    import concourse.bacc as bacc
    import concourse.mybir as mybir
    import concourse.tile as tile

    nc = bacc.Bacc("TRN2", target_bir_lowering=False, debug=False,
                   num_devices=N_CORES)
    xk = nc.dram_tensor("xk", [128, KTX, PIX_PER_CORE], mybir.dt.float16,
                        kind="ExternalInput").ap()
    wk = nc.dram_tensor("wk", [128, KTW, COUT], mybir.dt.float16,
                        kind="ExternalInput").ap()
    out = nc.dram_tensor("out", [PIX_PER_CORE, COUT], mybir.dt.float32,
                         kind="ExternalOutput").ap()

    with tile.TileContext(nc) as tc:
        with (
            tc.tile_pool(name="sbuf", bufs=1) as pool,
            tc.tile_pool(name="psum", bufs=1, space="PSUM") as psum_pool,
        ):
            xt = pool.tile([128, KTX, PIX_PER_CORE], mybir.dt.float16, name="xt")
            wt = pool.tile([128, KTW, COUT], mybir.dt.float16, name="wt")
            # One balanced 160KB DMA per DMA-capable engine (sync, scalar,
            # gpsimd): issue + first-byte + transfer overlap across the three
            # queues, and Tile's subtile deps let the hi-part matmuls run
            # while the lo-part x tiles are still landing. (Measured: finer
            # splits only add per-issue overhead — same-engine transfers
            # serialize.)
            nc.sync.dma_start(xt[:, 0:KTW, :], xk[:, 0:KTW, :])
            nc.scalar.dma_start(wt[:, :, :], wk[:, :, :])
            nc.gpsimd.dma_start(xt[:, KTW:KTX, :], xk[:, KTW:KTX, :])

            acc = psum_pool.tile([PIX_PER_CORE, COUT], mybir.dt.float32,
                                 name="acc")
            for r in range(KTX):
                nc.tensor.matmul(acc[:, :], xt[:, r, :], wt[:, r % KTW, :],
                                 start=(r == 0), stop=(r == KTX - 1))
            _epilogue_and_out(nc, mybir, pool, acc, out)

    nc.compile()
    return nc


def _build_fp16_raw_program():
    """Same dataflow as _build_fp16_program but hand-synchronized raw Bass
    (no TileContext): drops Tile's kernel-tail drain + two all-engine
    barriers + semaphore-clear pass."""
    import concourse.bacc as bacc
    import concourse.bass as bass
    import concourse.mybir as mybir

    nc = bacc.Bacc("TRN2", target_bir_lowering=False, debug=False,
                   num_devices=N_CORES)
    xk = nc.dram_tensor("xk", [128, KTX, PIX_PER_CORE], mybir.dt.float16,
                        kind="ExternalInput").ap()
    wk = nc.dram_tensor("wk", [128, KTW, COUT], mybir.dt.float16,
                        kind="ExternalInput").ap()
    out = nc.dram_tensor("out", [PIX_PER_CORE, COUT], mybir.dt.float32,
                         kind="ExternalOutput").ap()

    half = PIX_PER_CORE // 2
    with (
        nc.sbuf_tensor([128, KTX, PIX_PER_CORE], mybir.dt.float16) as xt,
        nc.sbuf_tensor([128, KTW, COUT], mybir.dt.float16) as wt,
        nc.sbuf_tensor([PIX_PER_CORE, COUT], mybir.dt.float32) as res,
        nc.psum_tensor([PIX_PER_CORE, COUT], mybir.dt.float32) as acc,
        nc.semaphore("s_x1") as s_x1,
        nc.semaphore("s_w") as s_w,
        nc.semaphore("s_x2") as s_x2,
        nc.semaphore("s_mm") as s_mm,
        nc.semaphore("s_v") as s_v,
        nc.semaphore("s_out") as s_out,
        nc.Block() as block,
    ):
        @block.sync
        def _(sync):
            sync.dma_start(xt[:, 0:KTW, :], xk[:, 0:KTW, :]).then_inc(s_x1, 16)
            sync.wait_ge(s_v, 3)
            sync.dma_start(out[0:half, :], res[0:half, :]).then_inc(s_out, 16)
            sync.wait_ge(s_out, 32)

        @block.scalar
        def _(scalar):
            scalar.dma_start(wt[:, :, :], wk[:, :, :]).then_inc(s_w, 16)
            scalar.wait_ge(s_v, 3)
            scalar.dma_start(out[half:PIX_PER_CORE, :],
                             res[half:PIX_PER_CORE, :]).then_inc(s_out, 16)

        @block.gpsimd
        def _(gpsimd):
            gpsimd.dma_start(xt[:, KTW:KTX, :],
                             xk[:, KTW:KTX, :]).then_inc(s_x2, 16)

        @block.tensor
        def _(tensor):
            tensor.wait_ge(s_x1, 16)
            tensor.wait_ge(s_w, 16)
            for r in range(KTW):
                nc.tensor.matmul(acc[:, :], xt[:, r, :], wt[:, r, :],
                                 start=(r == 0), stop=False)
            tensor.wait_ge(s_x2, 16)
            for r in range(KTW, KTX):
                mm = nc.tensor.matmul(acc[:, :], xt[:, r, :],
                                      wt[:, r % KTW, :],
                                      start=False, stop=(r == KTX - 1))
            mm.then_inc(s_mm, 1)

        @block.vector
        def _(vector):
            # DVE pipelines back-to-back ops without hazard checks, so each
            # dependent tensor_scalar must wait on the previous one's
            # completion sem (same pattern Tile generates).
            vector.wait_ge(s_mm, 1)
            nc.vector.tensor_scalar(res[:, :], acc[:, :], _INV_Q, _MAGIC,
                                    op0=mybir.AluOpType.mult,
                                    op1=mybir.AluOpType.add).then_inc(s_v, 1)
            vector.wait_ge(s_v, 1)
            nc.vector.tensor_scalar(res[:, :], res[:, :], _MAGIC, _LO,
                                    op0=mybir.AluOpType.subtract,
                                    op1=mybir.AluOpType.max).then_inc(s_v, 1)
            vector.wait_ge(s_v, 2)
            nc.vector.tensor_scalar(res[:, :], res[:, :], _HI, _INV_Q,
                                    op0=mybir.AluOpType.min,
                                    op1=mybir.AluOpType.mult).then_inc(s_v, 1)

    nc.compile()
    return nc


def _build_fp32_program():
    """Fallback: 5 double-pumped fp32 matmuls over zero-padded k = 640."""
    import concourse.bacc as bacc
    import concourse.mybir as mybir
    import concourse.tile as tile

    nc = bacc.Bacc("TRN2", target_bir_lowering=False, debug=False,
                   num_devices=N_CORES)
    xk = nc.dram_tensor("xk", [128, KT32, PIX_PER_CORE], mybir.dt.float32,
                        kind="ExternalInput").ap()
    wk = nc.dram_tensor("wk", [128, KT32, COUT], mybir.dt.float32,
                        kind="ExternalInput").ap()
    out = nc.dram_tensor("out", [PIX_PER_CORE, COUT], mybir.dt.float32,
                         kind="ExternalOutput").ap()

    with tile.TileContext(nc) as tc:
        with (
            tc.tile_pool(name="sbuf", bufs=1) as pool,
            tc.tile_pool(name="psum", bufs=1, space="PSUM") as psum_pool,
        ):
            xt = pool.tile([128, KT32, PIX_PER_CORE], mybir.dt.float32, name="xt")
            wt = pool.tile([128, KT32, COUT], mybir.dt.float32, name="wt")
            nc.sync.dma_start(xt[:, :, :], xk[:, :, :])
            nc.gpsimd.dma_start(wt[:, :, :], wk[:, :, :])

            acc = psum_pool.tile([PIX_PER_CORE, COUT], mybir.dt.float32,
                                 name="acc")
            for r in range(KT32):
                nc.tensor.matmul(acc[:, :], xt[:, r, :], wt[:, r, :],
                                 start=(r == 0), stop=(r == KT32 - 1))
            _epilogue_and_out(nc, mybir, pool, acc, out)

    nc.compile()
    return nc


def _quantize_inputs(x, w):
    """Reproduce the reference's fixed-point quantization bit-exactly."""
    xi = np.round(x.astype(np.float32) * (1 << IF)).astype(np.int64)
    xi = ((xi + (1 << 15)) & 0xFFFF) - (1 << 15)  # int16 two's-complement wrap

    wf = w.reshape(COUT, L).astype(np.float32)
    w_pos = np.clip(np.round(np.clip(wf, 0, None) * (1 << WF)), 0, 65535)
    w_neg = np.clip(np.round(np.abs(np.clip(wf, None, 0)) * (1 << WF)), 0, 65535)
    wi = (w_pos - w_neg).astype(np.int64)  # [COUT, L], l = (cin, ki, kj)
    return xi, wi


def _im2col(xi):
    """[B, CIN, H, W] int -> patches [P, L] with l = (cin, ki, kj) order."""
    xpad = np.zeros((B, CIN, H + 2 * PAD, W + 2 * PAD), dtype=xi.dtype)
    xpad[:, :, PAD:PAD + H, PAD:PAD + W] = xi
    cols = [xpad[:, :, ki:ki + H, kj:kj + W]
            for ki in range(K) for kj in range(K)]
    p = np.stack(cols, axis=2)  # [B, CIN, K*K, H, W]
    return p.reshape(B, L, H * W).transpose(0, 2, 1).reshape(B * H * W, L)


def _prepare(x, w):
    """Quantize + stage inputs; returns (program_key, builder, in_maps)."""
    x = np.asarray(x, dtype=np.float32)
    w = np.asarray(w, dtype=np.float32)

    xi, wi = _quantize_inputs(x, w)          # int64: [B,CIN,H,W], [COUT, L]
    patches = _im2col(xi)                    # [P, L] int64
    wmat = wi.T                              # [L, COUT] int64

    # fp16 path is exact iff |w_int| fits fp16's 11-bit mantissa (the x split
    # parts 256*xh in [-2^15, 2^15) and xl in [0, 256) are always exact).
    use_fp16 = np.abs(wi).max() <= 2048

    if use_fp16:
        xh = patches >> 8                    # floor division: [-128, 128)
        xl = patches & 0xFF                  # [0, 256)
        LP = KTW * 128                       # 640, zero-padded per group
        xe = np.zeros((KTX * 128, B * H * W), dtype=np.float16)
        xe[:L, :] = (xh.T * 256).astype(np.float16)          # hi group
        xe[LP:LP + L, :] = xl.T.astype(np.float16)           # lo group
        we = np.zeros((KTW * 128, COUT), dtype=np.float16)
        we[:L, :] = wmat.astype(np.float16)
        xtiles = np.ascontiguousarray(
            xe.reshape(KTX, 128, B * H * W).transpose(1, 0, 2))
        wtiles = np.ascontiguousarray(
            we.reshape(KTW, 128, COUT).transpose(1, 0, 2))
        key = "nc16"
        builder = _build_fp16_raw_program
    else:
        xe = np.zeros((KT32 * 128, B * H * W), dtype=np.float32)
        xe[:L, :] = patches.T.astype(np.float32)
        we = np.zeros((KT32 * 128, COUT), dtype=np.float32)
        we[:L, :] = wmat.astype(np.float32)
        xtiles = np.ascontiguousarray(
            xe.reshape(KT32, 128, B * H * W).transpose(1, 0, 2))
        wtiles = np.ascontiguousarray(
            we.reshape(KT32, 128, COUT).transpose(1, 0, 2))
        key = "nc32"
        builder = _build_fp32_program

    in_maps = []
    for core in range(N_CORES):
        p0 = core * PIX_PER_CORE
        in_maps.append({
            "xk": np.ascontiguousarray(xtiles[:, :, p0:p0 + PIX_PER_CORE]),
            "wk": wtiles,
        })
    return key, builder, in_maps


def kernel(x: np.ndarray, w: np.ndarray) -> np.ndarray:
    from concourse.bass_utils import run_bass_kernel_spmd

    key, builder, in_maps = _prepare(x, w)
    if key not in _CACHE:
        _CACHE[key] = builder()
    nc = _CACHE[key]

    results = run_bass_kernel_spmd(nc, in_maps, list(range(N_CORES))).results

    # Per-core shard: [128 pixels, COUT], pixels are (row, col) of half an image.
    out = np.empty((B, COUT, H, W), dtype=np.float32)
    for core in range(N_CORES):
        b, half = divmod(core, 2)
        r0 = half * ROWS_PER_CORE
        shard = results[core]["out"].reshape(ROWS_PER_CORE, W, COUT)
        out[b, :, r0:r0 + ROWS_PER_CORE, :] = shard.transpose(2, 0, 1)
    return out



# revision 2
# speedup vs baseline: 1.1849x; 1.1849x over previous
"""Trainium2 Bass kernel for nn_Conv2d_mvm (PUMA bit-sliced crossbar conv emulation).

Math identity
-------------
The reference emulates an analog crossbar MVM: inputs become 16-bit
two's-complement bit-streams, weights become 2-bit slices of the 16-bit
magnitudes of their pos/neg parts, and ADC = clip(round(analog), 0, 511).
Each analog column sum is at most 128*3 = 384 < 511 and every quantity is a
small exact integer held in f32, so the ADC is the identity and the whole
pipeline is linear in the bits/slices. Shift-add therefore reconstructs

    out[p, c] = quant( (x_int[p, :] . w_int[c, :]) / 2^24 )

with x_int = round(patch * 2^12) (int16 wrap),
w_int = clip(round(relu(w)*2^12), 0, 65535) - clip(round(relu(-w)*2^12), 0, 65535),
quant(v) = clip(round(v * 2^12), -2^15, 2^15-1) / 2^12  (round-half-even).

Device kernel
-------------
Data-parallel over the P = 1024 output pixels: each of 8 cores computes 128
pixels (half of one batch image) against the replicated [L=576, Cout=128]
integer weight matrix.

The PE's fp32 matmul is double-pumped, so the integer matmul runs in fp16,
which is exact here: |w_int| < 2048 fits fp16's 11-bit mantissa, and
x_int = 256*xh + xl splits into two fp16-exact factors folded into the
contraction dimension (10 zero-padded k-tiles sharing 5 weight k-tiles).

Schedule (measured on HW, NTFF traces):
- x (320KB fp16, one DMA, 2560B/partition descriptors) on the sync HWDGE
  queue; w as int16 (80KB) on the gpsimd SWDGE queue with an inline
  int16->fp16 cast (exact for |w|<=2047, verified on HW). The two HWDGE
  queues share one serial descriptor generator, so spreading input DMAs
  across HWDGE queues does NOT overlap their generation - SWDGE has its own.
- ~26 warm-up matmuls on a memset scratch tile run during the input-DMA
  wait so the PE HAM clock-gate (1.2 -> 2.4 GHz after ~3.4us of activity)
  is released by the time the 10 real matmuls run.
- Epilogue is ONE DVE op: tensor_scalar(out_i16, acc, 1/4096, +0) - the
  fp32->int16 output conversion is round-to-nearest-even + saturating
  (verified on HW), which IS the reference quantizer. The host rescales.
- The output store (32KB int16) issues on sync with a completion semaphore
  pinned to #205 but NO wait: the block-end barrier only drains descriptor
  generation, and the NEFF's multi-microsecond semaphore-clear postamble
  guarantees the data + sem-inc land long before the NEFF retires (and
  sem 205 is cleared ~4us after the inc lands, so re-execution is clean).
"""

import numpy as np

# Problem constants (hardcoded: kernel.py must be self-contained).
B, CIN, H, W = 4, 64, 16, 16
COUT = 128
K, PAD = 3, 1
IF = 12           # input frac bits
WF = 12           # weight frac bits
ACM_FRAC = 12
L = CIN * K * K   # 576
N_CORES = 8
ROWS_PER_CORE = H // 2            # 8 pixel rows per core
PIX_PER_CORE = ROWS_PER_CORE * W  # 128
KTW = 5                           # weight k-tiles (640 = 5*128, zero-padded)
KTX = 2 * KTW                     # fp16 x k-tiles: 5 hi-part + 5 lo-part
KT32 = 5                          # fp32 k-tiles (640 = 5*128, zero-padded)
N_WARM = 26                       # PE warm-up matmuls during the input DMA

_CACHE = {}

_MAGIC = float(np.float32(1.5 * 2 ** 23))  # f32 RNE rounding constant
_INV_Q = 1.0 / (1 << ACM_FRAC)
_LO = float(-(1 << 15))
_HI = float((1 << 15) - 1)


def _build_fp16_program():
    """Raw-Bass fp16 program: see module docstring for the schedule."""
    import concourse.bacc as bacc
    import concourse.mybir as mybir

    nc = bacc.Bacc("TRN2", target_bir_lowering=False, debug=False,
                   num_devices=N_CORES)
    xk = nc.dram_tensor("xk", [128, KTX, PIX_PER_CORE], mybir.dt.float16,
                        kind="ExternalInput").ap()
    wk16 = nc.dram_tensor("wk16", [128, KTW, COUT], mybir.dt.int16,
                          kind="ExternalInput").ap()
    out = nc.dram_tensor("out", [PIX_PER_CORE, COUT], mybir.dt.int16,
                         kind="ExternalOutput").ap()

    # Store-completion sem: never waited on; #205 is cleared near the END of
    # the NEFF postamble's per-engine sweep, ~4us after the inc can land.
    s_st = nc.alloc_semaphore("s_st", num=205)
    with (
        nc.sbuf_tensor([128, KTX, PIX_PER_CORE], mybir.dt.float16) as xt,
        nc.sbuf_tensor([128, KTW, COUT], mybir.dt.float16) as wt,
        nc.sbuf_tensor([128, 128], mybir.dt.float16) as warm,
        nc.sbuf_tensor([PIX_PER_CORE, COUT], mybir.dt.int16) as r16,
        nc.psum_tensor([PIX_PER_CORE, COUT], mybir.dt.float32) as acc,
        nc.psum_tensor([PIX_PER_CORE, COUT], mybir.dt.float32) as scratch,
        nc.semaphore("s_x") as s_x,
        nc.semaphore("s_w") as s_w,
        nc.semaphore("s_warm") as s_warm,
        nc.semaphore("s_mm") as s_mm,
        nc.semaphore("s_v") as s_v,
        nc.Block() as block,
    ):
        @block.sync
        def _(sync):
            sync.dma_start(xt[:, :, :], xk[:, :, :]).then_inc(s_x, 16)
            sync.wait_ge(s_v, 1)
            sync.dma_start(out[:, :], r16[:, :]).then_inc(s_st, 16)

        @block.gpsimd
        def _(gpsimd):
            # SWDGE inline cast int16 -> fp16 (exact: |w_int| <= 2047)
            gpsimd.dma_start(wt[:, :, :], wk16[:, :, :]).then_inc(s_w, 16)

        @block.vector
        def _(vector):
            vector.memset(warm[:, :], 0.0).then_inc(s_warm, 1)
            vector.wait_ge(s_mm, 1)
            # fp32 -> int16 is RNE + saturating: exactly the reference
            # quantizer (clip(round(acc/2^12), -2^15, 2^15-1)).
            nc.vector.tensor_scalar(r16[:, :], acc[:, :], _INV_Q, 0.0,
                                    op0=mybir.AluOpType.mult,
                                    op1=mybir.AluOpType.add).then_inc(s_v, 1)

        @block.tensor
        def _(tensor):
            tensor.wait_ge(s_warm, 1)
            for _i in range(N_WARM):
                nc.tensor.matmul(scratch[:, :], warm[:, :], warm[:, :],
                                 start=True, stop=True)
            tensor.wait_ge(s_w, 16)
            tensor.wait_ge(s_x, 16)
            for r in range(KTX):
                mm = nc.tensor.matmul(acc[:, :], xt[:, r, :],
                                      wt[:, r % KTW, :],
                                      start=(r == 0), stop=(r == KTX - 1))
            mm.then_inc(s_mm, 1)

    nc.compile()
    return nc


def _build_fp32_program():
    """Fallback: 5 double-pumped fp32 matmuls over zero-padded k = 640."""
    import concourse.bacc as bacc
    import concourse.mybir as mybir
    import concourse.tile as tile

    nc = bacc.Bacc("TRN2", target_bir_lowering=False, debug=False,
                   num_devices=N_CORES)
    xk = nc.dram_tensor("xk", [128, KT32, PIX_PER_CORE], mybir.dt.float32,
                        kind="ExternalInput").ap()
    wk = nc.dram_tensor("wk", [128, KT32, COUT], mybir.dt.float32,
                        kind="ExternalInput").ap()
    out = nc.dram_tensor("out", [PIX_PER_CORE, COUT], mybir.dt.float32,
                         kind="ExternalOutput").ap()

    with tile.TileContext(nc) as tc:
        with (
            tc.tile_pool(name="sbuf", bufs=1) as pool,
            tc.tile_pool(name="psum", bufs=1, space="PSUM") as psum_pool,
        ):
            xt = pool.tile([128, KT32, PIX_PER_CORE], mybir.dt.float32, name="xt")
            wt = pool.tile([128, KT32, COUT], mybir.dt.float32, name="wt")
            nc.sync.dma_start(xt[:, :, :], xk[:, :, :])
            nc.gpsimd.dma_start(wt[:, :, :], wk[:, :, :])

            acc = psum_pool.tile([PIX_PER_CORE, COUT], mybir.dt.float32,
                                 name="acc")
            for r in range(KT32):
                nc.tensor.matmul(acc[:, :], xt[:, r, :], wt[:, r, :],
                                 start=(r == 0), stop=(r == KT32 - 1))
            res = pool.tile([PIX_PER_CORE, COUT], mybir.dt.float32, name="res")
            nc.vector.tensor_scalar(res[:, :], acc[:, :], _INV_Q, _MAGIC,
                                    op0=mybir.AluOpType.mult,
                                    op1=mybir.AluOpType.add)
            nc.vector.tensor_scalar(res[:, :], res[:, :], _MAGIC, _LO,
                                    op0=mybir.AluOpType.subtract,
                                    op1=mybir.AluOpType.max)
            nc.vector.tensor_scalar(res[:, :], res[:, :], _HI, _INV_Q,
                                    op0=mybir.AluOpType.min,
                                    op1=mybir.AluOpType.mult)
            nc.sync.dma_start(out[:, :], res[:, :])

    nc.compile()
    return nc


def _quantize_inputs(x, w):
    """Reproduce the reference's fixed-point quantization bit-exactly."""
    xi = np.round(x.astype(np.float32) * (1 << IF)).astype(np.int64)
    xi = ((xi + (1 << 15)) & 0xFFFF) - (1 << 15)  # int16 two's-complement wrap

    wf = w.reshape(COUT, L).astype(np.float32)
    w_pos = np.clip(np.round(np.clip(wf, 0, None) * (1 << WF)), 0, 65535)
    w_neg = np.clip(np.round(np.abs(np.clip(wf, None, 0)) * (1 << WF)), 0, 65535)
    wi = (w_pos - w_neg).astype(np.int64)  # [COUT, L], l = (cin, ki, kj)
    return xi, wi


def _im2col(xi):
    """[B, CIN, H, W] int -> patches [P, L] with l = (cin, ki, kj) order."""
    xpad = np.zeros((B, CIN, H + 2 * PAD, W + 2 * PAD), dtype=xi.dtype)
    xpad[:, :, PAD:PAD + H, PAD:PAD + W] = xi
    cols = [xpad[:, :, ki:ki + H, kj:kj + W]
            for ki in range(K) for kj in range(K)]
    p = np.stack(cols, axis=2)  # [B, CIN, K*K, H, W]
    return p.reshape(B, L, H * W).transpose(0, 2, 1).reshape(B * H * W, L)


def _prepare(x, w):
    """Quantize + stage inputs; returns (program_key, builder, in_maps)."""
    x = np.asarray(x, dtype=np.float32)
    w = np.asarray(w, dtype=np.float32)

    xi, wi = _quantize_inputs(x, w)          # int64: [B,CIN,H,W], [COUT, L]
    patches = _im2col(xi)                    # [P, L] int64
    wmat = wi.T                              # [L, COUT] int64

    # fp16 path is exact iff |w_int| fits fp16's 11-bit mantissa (the x split
    # parts 256*xh in [-2^15, 2^15) and xl in [0, 256) are always exact).
    use_fp16 = np.abs(wi).max() <= 2047

    if use_fp16:
        xh = patches >> 8                    # floor division: [-128, 128)
        xl = patches & 0xFF                  # [0, 256)
        LP = KTW * 128                       # 640, zero-padded per group
        xe = np.zeros((KTX * 128, B * H * W), dtype=np.float16)
        xe[:L, :] = (xh.T * 256).astype(np.float16)          # hi group
        xe[LP:LP + L, :] = xl.T.astype(np.float16)           # lo group
        we = np.zeros((KTW * 128, COUT), dtype=np.int16)
        we[:L, :] = wmat.astype(np.int16)
        xtiles = np.ascontiguousarray(
            xe.reshape(KTX, 128, B * H * W).transpose(1, 0, 2))
        wtiles = np.ascontiguousarray(
            we.reshape(KTW, 128, COUT).transpose(1, 0, 2))
        key = "nc16"
        builder = _build_fp16_program
        in_maps = []
        for core in range(N_CORES):
            p0 = core * PIX_PER_CORE
            in_maps.append({
                "xk": np.ascontiguousarray(xtiles[:, :, p0:p0 + PIX_PER_CORE]),
                "wk16": wtiles,
            })
    else:
        xe = np.zeros((KT32 * 128, B * H * W), dtype=np.float32)
        xe[:L, :] = patches.T.astype(np.float32)
        we = np.zeros((KT32 * 128, COUT), dtype=np.float32)
        we[:L, :] = wmat.astype(np.float32)
        xtiles = np.ascontiguousarray(
            xe.reshape(KT32, 128, B * H * W).transpose(1, 0, 2))
        wtiles = np.ascontiguousarray(
            we.reshape(KT32, 128, COUT).transpose(1, 0, 2))
        key = "nc32"
        builder = _build_fp32_program
        in_maps = []
        for core in range(N_CORES):
            p0 = core * PIX_PER_CORE
            in_maps.append({
                "xk": np.ascontiguousarray(xtiles[:, :, p0:p0 + PIX_PER_CORE]),
                "wk": wtiles,
            })
    return key, builder, in_maps


def kernel(x: np.ndarray, w: np.ndarray) -> np.ndarray:
    from concourse.bass_utils import run_bass_kernel_spmd

    key, builder, in_maps = _prepare(x, w)
    if key not in _CACHE:
        _CACHE[key] = builder()
    nc = _CACHE[key]

    results = run_bass_kernel_spmd(nc, in_maps, list(range(N_CORES))).results

    # Per-core shard: [128 pixels, COUT], pixels are (row, col) of half an image.
    out = np.empty((B, COUT, H, W), dtype=np.float32)
    for core in range(N_CORES):
        b, half = divmod(core, 2)
        r0 = half * ROWS_PER_CORE
        shard = results[core]["out"]
        if shard.dtype == np.int16:  # device returns the int16 quantizer grid
            shard = shard.astype(np.float32) * _INV_Q
        shard = shard.reshape(ROWS_PER_CORE, W, COUT)
        out[b, :, r0:r0 + ROWS_PER_CORE, :] = shard.transpose(2, 0, 1)
    return out
